# revision 23
# baseline (speedup 1.0000x reference)
"""ButterflyLinear TRN2 kernel — 8-core data-parallel dense matmul.

The module  out = blockdiag(shuffle(blockdiag(x, stage1)) @ mix_w.T, stage2)
is a fixed linear map on the 2048-d feature axis.  We fold
stage1 ∘ shuffle ∘ mix ∘ stage2 into a single dense A [2048, 2048] on the
host (cheap block-wise einsums, fp64), then each NeuronCore computes
yT = A.T @ xT for its 2048-token shard: feature-major layout so the
contraction dim sits on SBUF partitions.  Operands are fp16 on the device
(FWL-fast weight loads, half the DMA bytes); accumulation is fp32 PSUM and
the output is fp32.

v2: the hot loop is ordered (m, k, n) so each stationary tile a[m,k] is
streamed against all G=4 moving token slabs; a post-build pass removes the
redundant per-matmul InstLdweights (the add-time split emits one per
matmul even when the weights AP is unchanged), cutting PE weight loads
4x.  x is double-buffered so back-to-back kernel iterations overlap.
"""

import sys

if "/opt/trn_rl_repo" not in sys.path:
    sys.path.insert(0, "/opt/trn_rl_repo")

import numpy as np

IN_F = 2048
OUT_F = 2048
BS = 64
NIB = IN_F // BS
NOB = OUT_F // BS
N_CORES = 8
TOK_PC = 2048  # tokens per core (16384 / 8)

P = 128
KT = IN_F // P  # 16 k-tiles
MT = OUT_F // P  # 16 m-tiles
NT = 512  # token tile (matmul moving dim)
NN = TOK_PC // NT  # 4 token tiles per core

_CACHE = {}


def _ap_key(pap):
    try:
        return (pap.memref, pap.offset, str(pap.ap), str(pap.dtype))
    except Exception:
        return None


def dedup_ldweights(nc):
    """Remove InstLdweights whose weights AP matches the previous PE weight
    load with no intervening clobber.  Matmuls were already split into
    (InstLdweights, InstMatmult(ldweights=False)) pairs at add time, so the
    later duplicates are pure overhead; dangling dependency references are
    remapped onto the duplicate's own matmul (same engine, later in program
    order — conservative for WAR on the SBUF weights tile)."""
    import concourse.mybir as mybir

    removed_total = 0
    pe = mybir.EngineType.PE
    for fn in nc.m.functions:
        for blk in fn.blocks:
            insts = list(blk.instructions)
            keep, removed_map = [], {}
            prev_key = None
            for idx, inst in enumerate(insts):
                tn = type(inst).__name__
                if tn == "InstLdweights":
                    key = _ap_key(inst.ins[0])
                    nxt = None
                    for j in range(idx + 1, len(insts)):
                        if insts[j].engine == pe:
                            nxt = insts[j]
                            break
                    if (
                        key is not None
                        and key == prev_key
                        and nxt is not None
                        and type(nxt).__name__ == "InstMatmult"
                        and not nxt.is_transpose
                    ):
                        nxt.merge_dependencies_from(inst)
                        removed_map[inst.name] = nxt.name
                        continue
                    prev_key = key
                elif inst.engine == pe:
                    if tn == "InstMatmult" and not inst.is_transpose:
                        pass  # plain matmul does not clobber loaded weights
                    elif tn in ("InstEventSemaphore", "InstDrain", "InstNop"):
                        pass  # sequencer-only
                    else:
                        prev_key = None
            # rebuild the list only if something was removed
            if removed_map:
                keep = [i for i in insts if i.name not in removed_map]
                blk.instructions = keep
                removed_total += len(removed_map)
                for fn2 in nc.m.functions:
                    for blk2 in fn2.blocks:
                        for inst2 in blk2.instructions:
                            inst2.remap_dependency_names(removed_map)
    return removed_total


def thin_pe_updates(nc):
    """The Tile framework puts a PE_* sem increment on every matmul, but
    consumers only ever wait on a handful of values (one per m-group).
    Drop the unconsumed increments and renumber the waited values (and the
    loop-reset add/sub-imm constants) so every release still fires on
    exactly the same matmul.  ~1000 serialized EVT_SEM writes removed per
    iteration."""
    from collections import defaultdict

    waited = defaultdict(set)
    incs = defaultdict(list)
    evs = defaultdict(list)
    for fn in nc.m.functions:
        for blk in fn.blocks:
            for inst in blk.instructions:
                si = inst.sync_info
                if si is None:
                    continue
                for w in si.on_wait or []:
                    if w.sync_type == "semaphore" and w.ant_name.startswith("PE_"):
                        waited[w.ant_name].add(w.wait_value or 0)
                for u in si.on_update or []:
                    if u.sync_type == "semaphore" and u.ant_name.startswith("PE_"):
                        if u.update_mode == "sem-inc":
                            incs[u.ant_name].append((inst, u))
                        else:
                            evs[u.ant_name].append((inst, u))
    for sem, lst in incs.items():
        total = len(lst)
        W = sorted(v for v in waited.get(sem, ()) if v > 0)
        if total < 64 or not W or max(W) > total:
            continue
        if any((u.update_value or 1) != 1 for _, u in lst):
            continue
        ev = evs.get(sem, [])
        if not all(
            u.update_mode in ("sem-add-imm", "sem-sub-imm")
            and u.update_value == total
            for _, u in ev
        ):
            continue
        keep = set(W)
        c = 0
        kept = 0
        new_rank = {}
        for inst, u in lst:
            c += 1
            if c in keep:
                kept += 1
                new_rank[c] = kept
            else:
                si = inst.sync_info
                si.on_update = [x for x in (si.on_update or []) if x is not u]
        for fn in nc.m.functions:
            for blk in fn.blocks:
                for inst in blk.instructions:
                    si = inst.sync_info
                    if si is None:
                        continue
                    for w in si.on_wait or []:
                        if (
                            w.sync_type == "semaphore"
                            and w.ant_name == sem
                            and (w.wait_value or 0) > 0
                        ):
                            w.wait_value = new_rank[w.wait_value]
        for _inst, u in ev:
            u.update_value = kept
    return


def _build(
    repeats: int = 1,
    loop_iters: int = 0,
    hoist_in_dma: bool = False,
    drop_y: bool = False,
    drop_copy: bool = False,
):
    """Build + compile the per-core Bass program (SPMD, same on all cores).

    loop_iters > 0 wraps the body in a hardware For_i loop (timing builds).
    hoist_in_dma/drop_y/drop_copy are perf-bisection knobs (timing only)."""
    import contextlib

    import concourse.mybir as mybir
    import concourse.tile as tile
    from concourse import bacc

    nc = bacc.Bacc(None, target_bir_lowering=False, debug=False)
    f32 = mybir.dt.float32
    f16 = mybir.dt.float16
    bf16 = mybir.dt.bfloat16

    # A is host-pretiled to [m, p, k, c] so each per-m load is 4KB-contiguous
    # per partition.  x is [in_feat, tokens] (feature-major).  y is stored
    # bf16 (host upcasts) to halve the output DMA traffic.
    a_ext = nc.declare_dram_parameter("a", [MT, P, KT, P], f16, isOutput=False)
    x_ext = nc.declare_dram_parameter("x", [IN_F, TOK_PC], f16, isOutput=False)
    y_ext = nc.declare_dram_parameter("y", [OUT_F, TOK_PC], bf16, isOutput=True)

    x_v = x_ext[:].rearrange("(k p) t -> p k t", p=P)

    with tile.TileContext(nc) as tc:
        with (
            tc.tile_pool(name="a_pool", bufs=1) as a_pool,
            tc.tile_pool(name="x_pool", bufs=2) as x_pool,
            tc.tile_pool(name="o_pool", bufs=3) as o_pool,
            tc.tile_pool(name="ps", bufs=2, space="PSUM") as ps_pool,
        ):
            def load_inputs():
                # Cold-start critical path: the PE consumes one k-group
                # (all 4 slabs at k) every ~0.9-1µs, so x streams k-major
                # across BOTH HWDGE rings (slabs 0/1 on sync, 2/3 on
                # scalar).  A rides the scalar ring: tile 0 first (needed
                # at t=0), the rest after x since tile m isn't needed
                # until m k-loops in.
                a_t = [
                    a_pool.tile([P, KT, P], f16, tag=f"a{m}", name=f"at{m}")
                    for m in range(MT)
                ]
                x_t = [
                    x_pool.tile([P, KT, NT], f16, tag=f"x{n}", name=f"xt{n}")
                    for n in range(NN)
                ]
                # a0 is on the first-matmul critical path: split it across
                # both rings so it lands in half the time
                nc.sync.dma_start(a_t[0][:, : KT // 2], a_ext[0][:, : KT // 2])
                nc.scalar.dma_start(a_t[0][:, KT // 2 :], a_ext[0][:, KT // 2 :])
                for k in range(KT):
                    for n in range(NN):
                        eng = nc.sync if n < 2 else nc.scalar
                        eng.dma_start(
                            x_t[n][:, k, :], x_v[:, k, n * NT : (n + 1) * NT]
                        )
                # balance the remaining A tiles across both rings; their
                # deadlines (tile m by m k-loops in) are loose
                for m in range(1, MT):
                    eng = nc.sync if m < 8 else nc.scalar
                    eng.dma_start(a_t[m][:], a_ext[m])
                return a_t, x_t

            if hoist_in_dma:
                a_t, x_t = load_inputs()
            loop_cm = (
                tc.For_i(0, loop_iters, 1, hint_engines=(mybir.EngineType.PE,))
                if loop_iters
                else contextlib.nullcontext()
            )
            with loop_cm:
                for _rep in range(repeats):
                    if not hoist_in_dma:
                        a_t, x_t = load_inputs()
                    for m in range(MT):
                        # one 4-bank PSUM tile per m; each matmul writes a
                        # bank-aligned 512-col slice
                        ps = ps_pool.tile([P, NN * NT], f32, tag="ps")
                        for k, n in [(k, n) for k in range(KT) for n in range(NN)]:
                            nc.tensor.matmul(
                                ps[:, n * NT : (n + 1) * NT],
                                a_t[m][:, k, :],
                                x_t[n][:, k, :],
                                start=(k == 0),
                                stop=(k == KT - 1),
                            )
                        if drop_copy:
                            continue
                        o_t = o_pool.tile([P, NN * NT], bf16, tag="o")
                        nc.vector.tensor_copy(o_t[:], ps[:])
                        if drop_y:
                            continue
                        # gpsimd SWDGE ring: output stores must not share
                        # the sync FIFO with X-slab loads
                        nc.gpsimd.dma_start(
                            y_ext[m * P : (m + 1) * P, :], o_t[:]
                        )
    n_removed = dedup_ldweights(nc)
    assert n_removed >= repeats * MT * KT * (NN - 1) // 2, n_removed
    thin_pe_updates(nc)
    nc.compile()
    return nc


def get_nc(repeats: int = 1, loop_iters: int = 0, **kw):
    key = ("nc", repeats, loop_iters, tuple(sorted(kw.items())))
    if key not in _CACHE:
        _CACHE[key] = _build(repeats, loop_iters, **kw)
    return _CACHE[key]


def compose_A(stage1: np.ndarray, stage2: np.ndarray, mix_w: np.ndarray) -> np.ndarray:
    """Fold stage1 ∘ shuffle ∘ mix ∘ stage2 into one dense [IN_F, OUT_F] map."""
    j = np.arange(IN_F)
    sig = (j % BS) * NIB + j // BS  # shuffle: h2[:, j] = h1[:, sig(j)]
    Ms = np.empty((IN_F, OUT_F), dtype=np.float64)
    Ms[sig, :] = mix_w.T.astype(np.float64)  # y = h1 @ Ms
    A_mid = np.einsum(
        "gcd,gdo->gco",
        stage1.reshape(NIB, BS, BS).astype(np.float64),
        Ms.reshape(NIB, BS, OUT_F),
    ).reshape(IN_F, OUT_F)
    A = np.einsum(
        "igc,gcd->igd",
        A_mid.reshape(IN_F, NOB, BS),
        stage2.reshape(NOB, BS, BS).astype(np.float64),
    ).reshape(IN_F, OUT_F)
    return A.astype(np.float32)


def tile_A(A: np.ndarray) -> np.ndarray:
    """[IN_F, OUT_F] fp32 -> [m, p, k, c] fp16 device layout."""
    return np.ascontiguousarray(
        A.reshape(KT, P, MT, P).transpose(2, 1, 0, 3).astype(np.float16)
    )


def make_in_maps(x, stage1, stage2, mix_w):
    A = compose_A(np.asarray(stage1), np.asarray(stage2), np.asarray(mix_w))
    A_dev = tile_A(A)
    x_flat = np.ascontiguousarray(np.asarray(x), dtype=np.float32).reshape(-1, IN_F)
    in_maps = []
    for c in range(N_CORES):
        shard = x_flat[c * TOK_PC : (c + 1) * TOK_PC, :]
        xT = np.ascontiguousarray(shard.T.astype(np.float16))
        in_maps.append({"a": A_dev, "x": xT})
    return in_maps


def assemble_output(results, batch_shape):
    y_flat = np.empty((N_CORES * TOK_PC, OUT_F), dtype=np.float32)
    for c in range(N_CORES):
        y_flat[c * TOK_PC : (c + 1) * TOK_PC, :] = results[c]["y"].T.astype(
            np.float32
        )
    return y_flat.reshape(*batch_shape, OUT_F)


def kernel(x, stage1, stage2, mix_w):
    from concourse.bass_utils import run_bass_kernel_spmd

    batch_shape = np.asarray(x).shape[:-1]
    nc = get_nc()
    in_maps = make_in_maps(x, stage1, stage2, mix_w)
    res = run_bass_kernel_spmd(nc, in_maps, list(range(N_CORES)))
    return assemble_output(res.results, batch_shape)


# revision 25
# speedup vs baseline: 1.0197x; 1.0197x over previous
"""ButterflyLinear TRN2 kernel — 8-core data-parallel dense matmul.

The module  out = blockdiag(shuffle(blockdiag(x, stage1)) @ mix_w.T, stage2)
is a fixed linear map on the 2048-d feature axis.  We fold
stage1 ∘ shuffle ∘ mix ∘ stage2 into a single dense A [2048, 2048] on the
host (cheap block-wise einsums, fp64), then each NeuronCore computes
yT = A.T @ xT for its 2048-token shard: feature-major layout so the
contraction dim sits on SBUF partitions.  Operands are fp16 on the device
(FWL-fast weight loads, half the DMA bytes); accumulation is fp32 PSUM and
the output is fp32.

v2: the hot loop is ordered (m, k, n) so each stationary tile a[m,k] is
streamed against all G=4 moving token slabs; a post-build pass removes the
redundant per-matmul InstLdweights (the add-time split emits one per
matmul even when the weights AP is unchanged), cutting PE weight loads
4x.  x is double-buffered so back-to-back kernel iterations overlap.
"""

import sys

if "/opt/trn_rl_repo" not in sys.path:
    sys.path.insert(0, "/opt/trn_rl_repo")

import numpy as np

IN_F = 2048
OUT_F = 2048
BS = 64
NIB = IN_F // BS
NOB = OUT_F // BS
N_CORES = 8
TOK_PC = 2048  # tokens per core (16384 / 8)

P = 128
KT = IN_F // P  # 16 k-tiles
MT = OUT_F // P  # 16 m-tiles
NT = 512  # token tile (matmul moving dim)
NN = TOK_PC // NT  # 4 token tiles per core

_CACHE = {}


def _ap_key(pap):
    try:
        return (pap.memref, pap.offset, str(pap.ap), str(pap.dtype))
    except Exception:
        return None


def dedup_ldweights(nc):
    """Remove InstLdweights whose weights AP matches the previous PE weight
    load with no intervening clobber.  Matmuls were already split into
    (InstLdweights, InstMatmult(ldweights=False)) pairs at add time, so the
    later duplicates are pure overhead; dangling dependency references are
    remapped onto the duplicate's own matmul (same engine, later in program
    order — conservative for WAR on the SBUF weights tile)."""
    import concourse.mybir as mybir

    removed_total = 0
    pe = mybir.EngineType.PE
    for fn in nc.m.functions:
        for blk in fn.blocks:
            insts = list(blk.instructions)
            keep, removed_map = [], {}
            prev_key = None
            for idx, inst in enumerate(insts):
                tn = type(inst).__name__
                if tn == "InstLdweights":
                    key = _ap_key(inst.ins[0])
                    nxt = None
                    for j in range(idx + 1, len(insts)):
                        if insts[j].engine == pe:
                            nxt = insts[j]
                            break
                    if (
                        key is not None
                        and key == prev_key
                        and nxt is not None
                        and type(nxt).__name__ == "InstMatmult"
                        and not nxt.is_transpose
                    ):
                        nxt.merge_dependencies_from(inst)
                        removed_map[inst.name] = nxt.name
                        continue
                    prev_key = key
                elif inst.engine == pe:
                    if tn == "InstMatmult" and not inst.is_transpose:
                        pass  # plain matmul does not clobber loaded weights
                    elif tn in ("InstEventSemaphore", "InstDrain", "InstNop"):
                        pass  # sequencer-only
                    else:
                        prev_key = None
            # rebuild the list only if something was removed
            if removed_map:
                keep = [i for i in insts if i.name not in removed_map]
                blk.instructions = keep
                removed_total += len(removed_map)
                for fn2 in nc.m.functions:
                    for blk2 in fn2.blocks:
                        for inst2 in blk2.instructions:
                            inst2.remap_dependency_names(removed_map)
    return removed_total


def thin_pe_updates(nc):
    """The Tile framework puts a PE_* sem increment on every matmul, but
    consumers only ever wait on a handful of values (one per m-group).
    Drop the unconsumed increments and renumber the waited values (and the
    loop-reset add/sub-imm constants) so every release still fires on
    exactly the same matmul.  ~1000 serialized EVT_SEM writes removed per
    iteration."""
    from collections import defaultdict

    waited = defaultdict(set)
    incs = defaultdict(list)
    evs = defaultdict(list)
    for fn in nc.m.functions:
        for blk in fn.blocks:
            for inst in blk.instructions:
                si = inst.sync_info
                if si is None:
                    continue
                for w in si.on_wait or []:
                    if w.sync_type == "semaphore" and w.ant_name.startswith("PE_"):
                        waited[w.ant_name].add(w.wait_value or 0)
                for u in si.on_update or []:
                    if u.sync_type == "semaphore" and u.ant_name.startswith("PE_"):
                        if u.update_mode == "sem-inc":
                            incs[u.ant_name].append((inst, u))
                        else:
                            evs[u.ant_name].append((inst, u))
    for sem, lst in incs.items():
        total = len(lst)
        W = sorted(v for v in waited.get(sem, ()) if v > 0)
        if total < 64 or not W or max(W) > total:
            continue
        if any((u.update_value or 1) != 1 for _, u in lst):
            continue
        ev = evs.get(sem, [])
        if not all(
            u.update_mode in ("sem-add-imm", "sem-sub-imm")
            and u.update_value == total
            for _, u in ev
        ):
            continue
        keep = set(W)
        c = 0
        kept = 0
        new_rank = {}
        for inst, u in lst:
            c += 1
            if c in keep:
                kept += 1
                new_rank[c] = kept
            else:
                si = inst.sync_info
                si.on_update = [x for x in (si.on_update or []) if x is not u]
        for fn in nc.m.functions:
            for blk in fn.blocks:
                for inst in blk.instructions:
                    si = inst.sync_info
                    if si is None:
                        continue
                    for w in si.on_wait or []:
                        if (
                            w.sync_type == "semaphore"
                            and w.ant_name == sem
                            and (w.wait_value or 0) > 0
                        ):
                            w.wait_value = new_rank[w.wait_value]
        for _inst, u in ev:
            u.update_value = kept
    return


def _build(
    repeats: int = 1,
    loop_iters: int = 0,
    hoist_in_dma: bool = False,
    drop_y: bool = False,
    drop_copy: bool = False,
):
    """Build + compile the per-core Bass program (SPMD, same on all cores).

    loop_iters > 0 wraps the body in a hardware For_i loop (timing builds).
    hoist_in_dma/drop_y/drop_copy are perf-bisection knobs (timing only)."""
    import contextlib

    import concourse.mybir as mybir
    import concourse.tile as tile
    from concourse import bacc

    nc = bacc.Bacc(None, target_bir_lowering=False, debug=False)
    f32 = mybir.dt.float32
    f16 = mybir.dt.float16
    bf16 = mybir.dt.bfloat16

    # A is host-pretiled to [m, p, k, c] so each per-m load is 4KB-contiguous
    # per partition.  x is [in_feat, tokens] (feature-major).  y is stored
    # bf16 (host upcasts) to halve the output DMA traffic.
    a_ext = nc.declare_dram_parameter("a", [MT, P, KT, P], f16, isOutput=False)
    x_ext = nc.declare_dram_parameter("x", [IN_F, TOK_PC], f16, isOutput=False)
    y_ext = nc.declare_dram_parameter("y", [OUT_F, TOK_PC], bf16, isOutput=True)

    x_v = x_ext[:].rearrange("(k p) t -> p k t", p=P)

    with tile.TileContext(nc) as tc:
        with (
            tc.tile_pool(name="a_pool", bufs=1) as a_pool,
            tc.tile_pool(name="x_pool", bufs=2) as x_pool,
            tc.tile_pool(name="o_pool", bufs=3) as o_pool,
            tc.tile_pool(name="ps", bufs=2, space="PSUM") as ps_pool,
        ):
            def load_inputs():
                # Cold-start critical path: the PE consumes one k-group
                # (all 4 slabs at k) every ~0.9-1µs, so x streams k-major
                # across BOTH HWDGE rings (slabs 0/1 on sync, 2/3 on
                # scalar).  A rides the scalar ring: tile 0 first (needed
                # at t=0), the rest after x since tile m isn't needed
                # until m k-loops in.
                a_t = [
                    a_pool.tile([P, KT, P], f16, tag=f"a{m}", name=f"at{m}")
                    for m in range(MT)
                ]
                x_t = [
                    x_pool.tile([P, KT, NT], f16, tag=f"x{n}", name=f"xt{n}")
                    for n in range(NN)
                ]
                # a0 is on the first-matmul critical path: split it across
                # both rings so it lands in half the time
                nc.sync.dma_start(a_t[0][:, : KT // 2], a_ext[0][:, : KT // 2])
                nc.scalar.dma_start(a_t[0][:, KT // 2 :], a_ext[0][:, KT // 2 :])

                # x in 4-k-group chunks (4KB/partition descriptors amortize
                # the per-descriptor ring latency; chunk cadence still beats
                # the PE's ~4us-per-4-k-groups consumption).  Slabs 0/1 ride
                # sync with a1-a3 interleaved so early m-deadlines hold;
                # slabs 2/3 ride scalar followed by the late A tiles.
                KC = 4  # k-groups per chunk
                def xchunk(eng, n, c):
                    eng.dma_start(
                        x_t[n][:, c * KC : (c + 1) * KC, :],
                        x_v[:, c * KC : (c + 1) * KC, n * NT : (n + 1) * NT],
                    )
                for c in range(KT // KC):
                    xchunk(nc.sync, 0, c)
                    xchunk(nc.sync, 1, c)
                    if 1 + c < MT:
                        nc.sync.dma_start(a_t[1 + c][:], a_ext[1 + c])
                    xchunk(nc.scalar, 2, c)
                    xchunk(nc.scalar, 3, c)
                for m in range(1 + KT // KC, MT):
                    eng = nc.sync if m < 9 else nc.scalar
                    eng.dma_start(a_t[m][:], a_ext[m])
                return a_t, x_t

            if hoist_in_dma:
                a_t, x_t = load_inputs()
            loop_cm = (
                tc.For_i(0, loop_iters, 1, hint_engines=(mybir.EngineType.PE,))
                if loop_iters
                else contextlib.nullcontext()
            )
            with loop_cm:
                for _rep in range(repeats):
                    if not hoist_in_dma:
                        a_t, x_t = load_inputs()
                    for m in range(MT):
                        # one 4-bank PSUM tile per m; each matmul writes a
                        # bank-aligned 512-col slice
                        ps = ps_pool.tile([P, NN * NT], f32, tag="ps")
                        for k, n in [(k, n) for k in range(KT) for n in range(NN)]:
                            nc.tensor.matmul(
                                ps[:, n * NT : (n + 1) * NT],
                                a_t[m][:, k, :],
                                x_t[n][:, k, :],
                                start=(k == 0),
                                stop=(k == KT - 1),
                            )
                        if drop_copy:
                            continue
                        o_t = o_pool.tile([P, NN * NT], bf16, tag="o")
                        nc.vector.tensor_copy(o_t[:], ps[:])
                        if drop_y:
                            continue
                        # gpsimd SWDGE ring: output stores must not share
                        # the sync FIFO with X-slab loads.  The last store
                        # is the iteration tail — the sync ring is idle by
                        # then and faster.
                        eng = nc.sync if m == MT - 1 else nc.gpsimd
                        eng.dma_start(y_ext[m * P : (m + 1) * P, :], o_t[:])
    n_removed = dedup_ldweights(nc)
    assert n_removed >= repeats * MT * KT * (NN - 1) // 2, n_removed
    thin_pe_updates(nc)
    nc.compile()
    return nc


def get_nc(repeats: int = 1, loop_iters: int = 0, **kw):
    key = ("nc", repeats, loop_iters, tuple(sorted(kw.items())))
    if key not in _CACHE:
        _CACHE[key] = _build(repeats, loop_iters, **kw)
    return _CACHE[key]


def compose_A(stage1: np.ndarray, stage2: np.ndarray, mix_w: np.ndarray) -> np.ndarray:
    """Fold stage1 ∘ shuffle ∘ mix ∘ stage2 into one dense [IN_F, OUT_F] map."""
    j = np.arange(IN_F)
    sig = (j % BS) * NIB + j // BS  # shuffle: h2[:, j] = h1[:, sig(j)]
    Ms = np.empty((IN_F, OUT_F), dtype=np.float64)
    Ms[sig, :] = mix_w.T.astype(np.float64)  # y = h1 @ Ms
    A_mid = np.einsum(
        "gcd,gdo->gco",
        stage1.reshape(NIB, BS, BS).astype(np.float64),
        Ms.reshape(NIB, BS, OUT_F),
    ).reshape(IN_F, OUT_F)
    A = np.einsum(
        "igc,gcd->igd",
        A_mid.reshape(IN_F, NOB, BS),
        stage2.reshape(NOB, BS, BS).astype(np.float64),
    ).reshape(IN_F, OUT_F)
    return A.astype(np.float32)


def tile_A(A: np.ndarray) -> np.ndarray:
    """[IN_F, OUT_F] fp32 -> [m, p, k, c] fp16 device layout."""
    return np.ascontiguousarray(
        A.reshape(KT, P, MT, P).transpose(2, 1, 0, 3).astype(np.float16)
    )


def make_in_maps(x, stage1, stage2, mix_w):
    A = compose_A(np.asarray(stage1), np.asarray(stage2), np.asarray(mix_w))
    A_dev = tile_A(A)
    x_flat = np.ascontiguousarray(np.asarray(x), dtype=np.float32).reshape(-1, IN_F)
    in_maps = []
    for c in range(N_CORES):
        shard = x_flat[c * TOK_PC : (c + 1) * TOK_PC, :]
        xT = np.ascontiguousarray(shard.T.astype(np.float16))
        in_maps.append({"a": A_dev, "x": xT})
    return in_maps


def assemble_output(results, batch_shape):
    y_flat = np.empty((N_CORES * TOK_PC, OUT_F), dtype=np.float32)
    for c in range(N_CORES):
        y_flat[c * TOK_PC : (c + 1) * TOK_PC, :] = results[c]["y"].T.astype(
            np.float32
        )
    return y_flat.reshape(*batch_shape, OUT_F)


def kernel(x, stage1, stage2, mix_w):
    from concourse.bass_utils import run_bass_kernel_spmd

    batch_shape = np.asarray(x).shape[:-1]
    nc = get_nc()
    in_maps = make_in_maps(x, stage1, stage2, mix_w)
    res = run_bass_kernel_spmd(nc, in_maps, list(range(N_CORES)))
    return assemble_output(res.results, batch_shape)


# revision 30
# speedup vs baseline: 1.0233x; 1.0036x over previous
"""ButterflyLinear TRN2 kernel — 8-core data-parallel dense matmul.

The module  out = blockdiag(shuffle(blockdiag(x, stage1)) @ mix_w.T, stage2)
is a fixed linear map on the 2048-d feature axis.  We fold
stage1 ∘ shuffle ∘ mix ∘ stage2 into a single dense A [2048, 2048] on the
host (cheap block-wise einsums, fp64), then each NeuronCore computes
yT = A.T @ xT for its 2048-token shard: feature-major layout so the
contraction dim sits on SBUF partitions.  Operands are fp16 on the device
(FWL-fast weight loads, half the DMA bytes); accumulation is fp32 PSUM and
the output is fp32.

v2: the hot loop is ordered (m, k, n) so each stationary tile a[m,k] is
streamed against all G=4 moving token slabs; a post-build pass removes the
redundant per-matmul InstLdweights (the add-time split emits one per
matmul even when the weights AP is unchanged), cutting PE weight loads
4x.  x is double-buffered so back-to-back kernel iterations overlap.
"""

import sys

if "/opt/trn_rl_repo" not in sys.path:
    sys.path.insert(0, "/opt/trn_rl_repo")

import numpy as np

IN_F = 2048
OUT_F = 2048
BS = 64
NIB = IN_F // BS
NOB = OUT_F // BS
N_CORES = 8
TOK_PC = 2048  # tokens per core (16384 / 8)

P = 128
KT = IN_F // P  # 16 k-tiles
MT = OUT_F // P  # 16 m-tiles
NT = 512  # token tile (matmul moving dim)
NN = TOK_PC // NT  # 4 token tiles per core

_CACHE = {}


def _ap_key(pap):
    try:
        return (pap.memref, pap.offset, str(pap.ap), str(pap.dtype))
    except Exception:
        return None


def dedup_ldweights(nc):
    """Remove InstLdweights whose weights AP matches the previous PE weight
    load with no intervening clobber.  Matmuls were already split into
    (InstLdweights, InstMatmult(ldweights=False)) pairs at add time, so the
    later duplicates are pure overhead; dangling dependency references are
    remapped onto the duplicate's own matmul (same engine, later in program
    order — conservative for WAR on the SBUF weights tile)."""
    import concourse.mybir as mybir

    removed_total = 0
    pe = mybir.EngineType.PE
    for fn in nc.m.functions:
        for blk in fn.blocks:
            insts = list(blk.instructions)
            keep, removed_map = [], {}
            prev_key = None
            for idx, inst in enumerate(insts):
                tn = type(inst).__name__
                if tn == "InstLdweights":
                    key = _ap_key(inst.ins[0])
                    nxt = None
                    for j in range(idx + 1, len(insts)):
                        if insts[j].engine == pe:
                            nxt = insts[j]
                            break
                    if (
                        key is not None
                        and key == prev_key
                        and nxt is not None
                        and type(nxt).__name__ == "InstMatmult"
                        and not nxt.is_transpose
                    ):
                        nxt.merge_dependencies_from(inst)
                        removed_map[inst.name] = nxt.name
                        continue
                    prev_key = key
                elif inst.engine == pe:
                    if tn == "InstMatmult" and not inst.is_transpose:
                        pass  # plain matmul does not clobber loaded weights
                    elif tn in ("InstEventSemaphore", "InstDrain", "InstNop"):
                        pass  # sequencer-only
                    else:
                        prev_key = None
            # rebuild the list only if something was removed
            if removed_map:
                keep = [i for i in insts if i.name not in removed_map]
                blk.instructions = keep
                removed_total += len(removed_map)
                for fn2 in nc.m.functions:
                    for blk2 in fn2.blocks:
                        for inst2 in blk2.instructions:
                            inst2.remap_dependency_names(removed_map)
    return removed_total


def thin_pe_updates(nc):
    """The Tile framework puts a PE_* sem increment on every matmul, but
    consumers only ever wait on a handful of values (one per m-group).
    Drop the unconsumed increments and renumber the waited values (and the
    loop-reset add/sub-imm constants) so every release still fires on
    exactly the same matmul.  ~1000 serialized EVT_SEM writes removed per
    iteration."""
    from collections import defaultdict

    waited = defaultdict(set)
    incs = defaultdict(list)
    evs = defaultdict(list)
    for fn in nc.m.functions:
        for blk in fn.blocks:
            for inst in blk.instructions:
                si = inst.sync_info
                if si is None:
                    continue
                for w in si.on_wait or []:
                    if w.sync_type == "semaphore" and w.ant_name.startswith("PE_"):
                        waited[w.ant_name].add(w.wait_value or 0)
                for u in si.on_update or []:
                    if u.sync_type == "semaphore" and u.ant_name.startswith("PE_"):
                        if u.update_mode == "sem-inc":
                            incs[u.ant_name].append((inst, u))
                        else:
                            evs[u.ant_name].append((inst, u))
    for sem, lst in incs.items():
        total = len(lst)
        W = sorted(v for v in waited.get(sem, ()) if v > 0)
        if total < 64 or not W or max(W) > total:
            continue
        if any((u.update_value or 1) != 1 for _, u in lst):
            continue
        ev = evs.get(sem, [])
        if not all(
            u.update_mode in ("sem-add-imm", "sem-sub-imm")
            and u.update_value == total
            for _, u in ev
        ):
            continue
        keep = set(W)
        c = 0
        kept = 0
        new_rank = {}
        for inst, u in lst:
            c += 1
            if c in keep:
                kept += 1
                new_rank[c] = kept
            else:
                si = inst.sync_info
                si.on_update = [x for x in (si.on_update or []) if x is not u]
        for fn in nc.m.functions:
            for blk in fn.blocks:
                for inst in blk.instructions:
                    si = inst.sync_info
                    if si is None:
                        continue
                    for w in si.on_wait or []:
                        if (
                            w.sync_type == "semaphore"
                            and w.ant_name == sem
                            and (w.wait_value or 0) > 0
                        ):
                            w.wait_value = new_rank[w.wait_value]
        for _inst, u in ev:
            u.update_value = kept
    return


def _build(
    repeats: int = 1,
    loop_iters: int = 0,
    hoist_in_dma: bool = False,
    drop_y: bool = False,
    drop_copy: bool = False,
):
    """Build + compile the per-core Bass program (SPMD, same on all cores).

    loop_iters > 0 wraps the body in a hardware For_i loop (timing builds).
    hoist_in_dma/drop_y/drop_copy are perf-bisection knobs (timing only)."""
    import contextlib

    import concourse.mybir as mybir
    import concourse.tile as tile
    from concourse import bacc

    nc = bacc.Bacc(None, target_bir_lowering=False, debug=False)
    f32 = mybir.dt.float32
    f16 = mybir.dt.float16
    bf16 = mybir.dt.bfloat16

    # A is host-pretiled to [pair, p, 2, k, c] so each m-pair load is one
    # 8KB-contiguous-per-partition descriptor.  x is [in_feat, tokens]
    # (feature-major).  y is stored bf16 (host upcasts) to halve the output
    # DMA traffic.
    a_ext = nc.declare_dram_parameter(
        "a", [MT // 2, P, 2, KT, P], f16, isOutput=False
    )
    x_ext = nc.declare_dram_parameter("x", [IN_F, TOK_PC], f16, isOutput=False)
    y_ext = nc.declare_dram_parameter("y", [OUT_F, TOK_PC], bf16, isOutput=True)

    x_v = x_ext[:].rearrange("(k p) t -> p k t", p=P)

    with tile.TileContext(nc) as tc:
        with (
            tc.tile_pool(name="a_pool", bufs=1) as a_pool,
            tc.tile_pool(name="x_pool", bufs=2) as x_pool,
            tc.tile_pool(name="o_pool", bufs=3) as o_pool,
            tc.tile_pool(name="ps", bufs=2, space="PSUM") as ps_pool,
        ):
            def load_inputs():
                # Cold-start critical path: the PE consumes one k-group
                # (all 4 slabs at k) every ~1µs.  x streams in 4-k-group
                # chunks (4KB/partition descriptors amortize per-descriptor
                # ring latency) across BOTH HWDGE rings; A rides along as
                # 8KB/partition m-pair descriptors sequenced so each tile
                # lands before its m-loop starts.
                ap_t = [
                    a_pool.tile([P, 2, KT, P], f16, tag=f"ap{i}", name=f"apt{i}")
                    for i in range(MT // 2)
                ]
                a_t = []
                for i in range(MT // 2):
                    a_t.extend([ap_t[i][:, 0], ap_t[i][:, 1]])
                x_t = [
                    x_pool.tile([P, KT, NT], f16, tag=f"x{n}", name=f"xt{n}")
                    for n in range(NN)
                ]
                # a0 is on the first-matmul critical path: split it across
                # both rings so it lands in half the time; a1 follows early
                # on sync.
                nc.sync.dma_start(
                    ap_t[0][:, 0, : KT // 2], a_ext[0][:, 0, : KT // 2]
                )
                nc.scalar.dma_start(
                    ap_t[0][:, 0, KT // 2 :], a_ext[0][:, 0, KT // 2 :]
                )
                KC = 4  # k-groups per x chunk

                def xchunk(eng, n, c):
                    eng.dma_start(
                        x_t[n][:, c * KC : (c + 1) * KC, :],
                        x_v[:, c * KC : (c + 1) * KC, n * NT : (n + 1) * NT],
                    )

                xchunk(nc.sync, 0, 0)
                xchunk(nc.sync, 1, 0)
                nc.sync.dma_start(ap_t[0][:, 1], a_ext[0][:, 1])  # a1
                xchunk(nc.sync, 0, 1)
                xchunk(nc.sync, 1, 1)
                xchunk(nc.sync, 0, 2)
                xchunk(nc.sync, 1, 2)
                nc.sync.dma_start(ap_t[1][:], a_ext[1])  # a2,a3
                xchunk(nc.sync, 0, 3)
                xchunk(nc.sync, 1, 3)
                for i in (2, 3, 4):
                    nc.sync.dma_start(ap_t[i][:], a_ext[i])
                for c in range(KT // KC):
                    xchunk(nc.scalar, 2, c)
                    xchunk(nc.scalar, 3, c)
                for i in (5, 6, 7):
                    nc.scalar.dma_start(ap_t[i][:], a_ext[i])
                return a_t, x_t

            if hoist_in_dma:
                a_t, x_t = load_inputs()
            loop_cm = (
                tc.For_i(0, loop_iters, 1, hint_engines=(mybir.EngineType.PE,))
                if loop_iters
                else contextlib.nullcontext()
            )
            with loop_cm:
                for _rep in range(repeats):
                    if not hoist_in_dma:
                        a_t, x_t = load_inputs()
                    for m in range(MT):
                        # one 4-bank PSUM tile per m; each matmul writes a
                        # bank-aligned 512-col slice
                        ps = ps_pool.tile([P, NN * NT], f32, tag="ps")
                        for k, n in [(k, n) for k in range(KT) for n in range(NN)]:
                            nc.tensor.matmul(
                                ps[:, n * NT : (n + 1) * NT],
                                a_t[m][:, k, :],
                                x_t[n][:, k, :],
                                start=(k == 0),
                                stop=(k == KT - 1),
                            )
                        if drop_copy:
                            continue
                        o_t = o_pool.tile([P, NN * NT], bf16, tag="o")
                        # split the drain between DVE and ACT (different
                        # PSUM banks) to halve its latency
                        H = NN * NT // 2
                        nc.vector.tensor_copy(o_t[:, :H], ps[:, :H])
                        nc.scalar.copy(o_t[:, H:], ps[:, H:])
                        if drop_y:
                            continue
                        # gpsimd SWDGE ring: output stores must not share
                        # the sync FIFO with X-slab loads.  The last store
                        # is the iteration tail — the sync ring is idle by
                        # then and faster.
                        eng = nc.sync if m == MT - 1 else nc.gpsimd
                        eng.dma_start(y_ext[m * P : (m + 1) * P, :], o_t[:])
    n_removed = dedup_ldweights(nc)
    assert n_removed >= repeats * MT * KT * (NN - 1) // 2, n_removed
    thin_pe_updates(nc)
    nc.compile()
    return nc


def get_nc(repeats: int = 1, loop_iters: int = 0, **kw):
    key = ("nc", repeats, loop_iters, tuple(sorted(kw.items())))
    if key not in _CACHE:
        _CACHE[key] = _build(repeats, loop_iters, **kw)
    return _CACHE[key]


def compose_A(stage1: np.ndarray, stage2: np.ndarray, mix_w: np.ndarray) -> np.ndarray:
    """Fold stage1 ∘ shuffle ∘ mix ∘ stage2 into one dense [IN_F, OUT_F] map."""
    j = np.arange(IN_F)
    sig = (j % BS) * NIB + j // BS  # shuffle: h2[:, j] = h1[:, sig(j)]
    Ms = np.empty((IN_F, OUT_F), dtype=np.float64)
    Ms[sig, :] = mix_w.T.astype(np.float64)  # y = h1 @ Ms
    A_mid = np.einsum(
        "gcd,gdo->gco",
        stage1.reshape(NIB, BS, BS).astype(np.float64),
        Ms.reshape(NIB, BS, OUT_F),
    ).reshape(IN_F, OUT_F)
    A = np.einsum(
        "igc,gcd->igd",
        A_mid.reshape(IN_F, NOB, BS),
        stage2.reshape(NOB, BS, BS).astype(np.float64),
    ).reshape(IN_F, OUT_F)
    return A.astype(np.float32)


def tile_A(A: np.ndarray) -> np.ndarray:
    """[IN_F, OUT_F] fp32 -> [pair, p, 2, k, c] fp16 device layout."""
    t = A.reshape(KT, P, MT, P).transpose(2, 1, 0, 3)  # [m, p, k, c]
    t = t.reshape(MT // 2, 2, P, KT, P).transpose(0, 2, 1, 3, 4)
    return np.ascontiguousarray(t.astype(np.float16))


def make_in_maps(x, stage1, stage2, mix_w):
    A = compose_A(np.asarray(stage1), np.asarray(stage2), np.asarray(mix_w))
    A_dev = tile_A(A)
    x_flat = np.ascontiguousarray(np.asarray(x), dtype=np.float32).reshape(-1, IN_F)
    in_maps = []
    for c in range(N_CORES):
        shard = x_flat[c * TOK_PC : (c + 1) * TOK_PC, :]
        xT = np.ascontiguousarray(shard.T.astype(np.float16))
        in_maps.append({"a": A_dev, "x": xT})
    return in_maps


def assemble_output(results, batch_shape):
    y_flat = np.empty((N_CORES * TOK_PC, OUT_F), dtype=np.float32)
    for c in range(N_CORES):
        y_flat[c * TOK_PC : (c + 1) * TOK_PC, :] = results[c]["y"].T.astype(
            np.float32
        )
    return y_flat.reshape(*batch_shape, OUT_F)


def kernel(x, stage1, stage2, mix_w):
    from concourse.bass_utils import run_bass_kernel_spmd

    batch_shape = np.asarray(x).shape[:-1]
    nc = get_nc()
    in_maps = make_in_maps(x, stage1, stage2, mix_w)
    res = run_bass_kernel_spmd(nc, in_maps, list(range(N_CORES)))
    return assemble_output(res.results, batch_shape)
